# revision 61
# baseline (speedup 1.0000x reference)
"""CrystalGraphEncoder (2x TransformerConv + 2x GATConv + LN + mean-pool + MLP)
as a Bass/Tile kernel on 8 Trainium2 NeuronCores.

Strategy: shard destination nodes across cores (edges sorted by dst). Per layer:
sharded dense matmuls -> fp16 src-side tables -> AllGather -> bulk dma_gather of
per-edge src rows -> dst-side per-edge values via one-hot (fp8) transposed
matmuls (no gather) -> DVE edge math -> one-hot scatter matmuls into PSUM ->
normalize + skip + residual + LN on-chip. The next layer's dense matmuls are
interleaved into the edge loop so they hide under the gathers. Mean-pool via
one-hot matmul + AllReduce; final MLP replicated.
"""
import numpy as np
import ml_dtypes

import concourse.bacc as bacc
import concourse.tile as tile
from concourse import bass, mybir
from concourse import bass_utils
from concourse.masks import make_identity

F16 = mybir.dt.float16
F32 = mybir.dt.float32
F8 = mybir.dt.float8e4
I16 = mybir.dt.int16
NP_F16 = np.float16
NP_F8 = ml_dtypes.float8_e4m3
P = 128

# problem constants (from the reference model)
N_NODES = 20000
IN_DIM = 92
HID = 256
OUT_DIM = 128
HEADS = 8
HDIM = 32
N_GRAPHS = 128
LN_EPS = 1e-5
C = 8  # cores
GATW = 256  # GAT gather row: hh only (512B); a_src recomputed per-edge on DVE
HAB = 12  # dense blocks covered by the first AllGather chunk (of NB=20)
AF = mybir.ActivationFunctionType


def _wrap_idxs(idx):
    """[n] int -> [128, n//16] int16 dma_gather index layout (16-partition wrap,
    replicated for the 8 Q7 cores)."""
    n = len(idx)
    assert n % 16 == 0
    w = idx.reshape(n // 16, 16).T.astype(np.int16)
    return np.ascontiguousarray(np.tile(w, (8, 1)))


def _edge_struct(src_row, dst_local, dst_core, NB, pad_kv):
    """Per-core gather/scatter arrays for one edge set (sorted by dst)."""
    blk = dst_local // P
    slot = dst_local % P
    key = dst_core * NB + blk
    # secondary sort by src row: each SDMA engine then reads ascending table
    # rows during the gather (better HBM bank behavior than random order)
    order = np.lexsort((src_row, key))
    src_s = src_row[order]
    slot_s = slot[order]
    counts = np.bincount(key, minlength=C * NB)
    NT = int(np.ceil(counts.max() / P))
    starts = np.concatenate([[0], np.cumsum(counts)])

    per_core = []
    for c in range(C):
        n_slots = NB * NT * P
        kv_idx = np.full(n_slots, pad_kv, dtype=np.int64)
        S = np.zeros((NB, P, NT * P), dtype=NP_F8)   # [edge_pos, j*P + slot]
        ST = np.zeros((NB, P, NT * P), dtype=NP_F8)  # [slot, j*P + edge_pos]
        for b in range(NB):
            k = c * NB + b
            s, e = starts[k], starts[k + 1]
            n = e - s
            pos = b * NT * P + np.arange(n)  # flat position j*128+p
            kv_idx[pos] = src_s[s:e]
            jj = np.arange(n) // P
            pp = np.arange(n) % P
            S[b, pp, jj * P + slot_s[s:e]] = 1.0
            ST[b, slot_s[s:e], jj * P + pp] = 1.0
        per_core.append(
            dict(
                kv_idx=_wrap_idxs(kv_idx),
                S=np.ascontiguousarray(S.reshape(NB * P, NT * P)),
                ST=np.ascontiguousarray(ST.reshape(NB * P, NT * P)),
            )
        )
    return NT, per_core


def host_prep(inputs):
    """Split + pad + sort everything on the host. Returns (meta, in_maps)."""
    x = np.asarray(inputs["x"], np.float32)
    ei = np.asarray(inputs["edge_index"], np.int64)
    batch = np.asarray(inputs["batch"], np.int64)
    N = x.shape[0]
    RPC = (N + C - 1) // C
    NB = (RPC + P - 1) // P
    NPC = NB * P

    core_of = np.minimum(np.arange(N) // RPC, C - 1)
    local_of = np.arange(N) - core_of * RPC
    # row in the AllGathered tables; the table is laid out as
    # [halfA: C x (HAB*P) rows | halfB: C x (NB-HAB)*P rows | zero row] so the
    # two AllGather chunks write contiguous regions
    HA = HAB * P
    HB = NPC - HA
    grow = np.where(
        local_of < HA,
        core_of * HA + local_of,
        C * HA + core_of * HB + (local_of - HA),
    )

    PAD_KV = C * NPC  # zero row of global tables

    src, dst = ei[0], ei[1]
    NTT, tconv = _edge_struct(grow[src], local_of[dst], core_of[dst], NB, PAD_KV)
    sl = np.arange(N, dtype=np.int64)
    src_g = np.concatenate([src, sl])
    dst_g = np.concatenate([dst, sl])
    NTG, gat = _edge_struct(grow[src_g], local_of[dst_g], core_of[dst_g], NB, PAD_KV)

    cnt = np.bincount(batch, minlength=N_GRAPHS).astype(np.float32)
    invcnt = (1.0 / np.maximum(cnt, 1.0)).reshape(N_GRAPHS, 1)

    def f16(a):
        return np.asarray(a, np.float32).astype(NP_F16)

    def pack_k(w):  # [K, N] -> [128, K//128 * N] (chunk-major)
        w = np.asarray(w, np.float32)
        K, Nc = w.shape
        assert K % P == 0
        return np.ascontiguousarray(
            w.reshape(K // P, P, Nc).transpose(1, 0, 2).reshape(P, -1)
        ).astype(NP_F16)

    wdict = dict(
        win=f16(inputs["Win"]),
        b_in=f16(np.asarray(inputs["b_in"]).reshape(1, HID)),
        w1=pack_k(inputs["W1"]),
        b1=f16(np.asarray(inputs["b1"]).reshape(1, 2 * HID)),
        w2=pack_k(inputs["W2"]),
        b2=f16(np.asarray(inputs["b2"]).reshape(1, OUT_DIM)),
        invcnt=invcnt.astype(np.float32),
        zeros_row=np.zeros((1, 2 * HID), NP_F16),
    )
    for t in range(2):
        wkv = np.concatenate(
            [np.asarray(inputs["Wk"][t]), np.asarray(inputs["Wv"][t])], axis=1
        )
        bkv = np.concatenate(
            [np.asarray(inputs["bk"][t]), np.asarray(inputs["bv"][t])]
        )
        wdict[f"wkv{t}"] = pack_k(wkv)
        wdict[f"bkv{t}"] = f16(bkv.reshape(1, 2 * HID))
        wdict[f"wq{t}"] = pack_k(inputs["Wq"][t])
        wdict[f"bq{t}"] = f16(np.asarray(inputs["bq"][t]).reshape(1, HID))
        wdict[f"wsk{t}"] = pack_k(
            np.asarray(inputs["Wskip"][t], np.float64) + np.eye(HID)
        )
        wdict[f"bsk{t}"] = f16(np.asarray(inputs["bskip"][t]).reshape(1, HID))
        wdict[f"wg{t}"] = pack_k(inputs["Wg"][t])
        wdict[f"bg{t}"] = f16(np.asarray(inputs["bg"][t]).reshape(1, HID))
        wdict[f"atts{t}"] = np.ascontiguousarray(
            np.broadcast_to(
                f16(np.asarray(inputs["att_src"][t]).reshape(1, HID)), (P, HID)
            )
        )
        wdict[f"attd{t}"] = np.ascontiguousarray(
            np.broadcast_to(
                f16(np.asarray(inputs["att_dst"][t]).reshape(1, HID)), (P, HID)
            )
        )

    ln_g = np.asarray(inputs["ln_g"], np.float32)
    ln_b = np.asarray(inputs["ln_b"], np.float32)
    ln_trivial = bool(np.all(ln_g == 1.0) and np.all(ln_b == 0.0))
    if not ln_trivial:
        for i in range(4):
            wdict[f"lng{i}"] = np.ascontiguousarray(
                np.broadcast_to(ln_g[i].reshape(1, HID).astype(NP_F16), (P, HID))
            )
            wdict[f"lnb{i}"] = np.ascontiguousarray(
                np.broadcast_to(ln_b[i].reshape(1, HID).astype(NP_F16), (P, HID))
            )

    in_maps = []
    for c in range(C):
        m = dict(wdict)
        lo, hi = c * RPC, min((c + 1) * RPC, N)
        xT = np.zeros((IN_DIM, NPC), np.float32)
        xT[:, 0 : hi - lo] = x[lo:hi].T
        m["xT"] = xT.astype(NP_F16)
        m["kvidx"] = tconv[c]["kv_idx"]
        m["S_t"] = tconv[c]["S"]
        m["ST_t"] = tconv[c]["ST"]
        m["gatidx"] = gat[c]["kv_idx"]
        m["S_g"] = gat[c]["S"]
        m["ST_g"] = gat[c]["ST"]
        Sp = np.zeros((NB, P, N_GRAPHS), dtype=NP_F8)
        ns = hi - lo
        bb = np.arange(ns) // P
        pp = np.arange(ns) % P
        Sp[bb, pp, batch[lo:hi]] = 1.0
        m["S_p"] = np.ascontiguousarray(Sp.reshape(NB * P, N_GRAPHS))
        in_maps.append(m)

    meta = dict(NB=NB, NPC=NPC, NTT=NTT, NTG=NTG, ln_trivial=ln_trivial)
    return meta, in_maps


DEBUG_DUMPS = False


def build_program(meta):
    NB = meta["NB"]
    NPC = meta["NPC"]
    NTT = meta["NTT"]
    NTG = meta["NTG"]
    ln_trivial = meta["ln_trivial"]
    TABN = C * NPC + 1
    NTMAX = max(NTT, NTG)

    nc = bacc.Bacc("TRN2", target_bir_lowering=False, debug=False, num_devices=C)

    def di(name, shape, dt):
        return nc.dram_tensor(name, shape, dt, kind="ExternalInput")

    xT_d = di("xT", [IN_DIM, NPC], F16)
    kvidx_d = di("kvidx", [P, NB * NTT * 8], I16)
    St_d = di("S_t", [NB * P, NTT * P], F8)
    STt_d = di("ST_t", [NB * P, NTT * P], F8)
    gatidx_d = di("gatidx", [P, NB * NTG * 8], I16)
    Sg_d = di("S_g", [NB * P, NTG * P], F8)
    STg_d = di("ST_g", [NB * P, NTG * P], F8)
    Sp_d = di("S_p", [NB * P, N_GRAPHS], F8)
    invcnt_d = di("invcnt", [N_GRAPHS, 1], F32)
    zeros_d = di("zeros_row", [1, 2 * HID], F16)
    win_d = di("win", [IN_DIM, HID], F16)
    bin_d = di("b_in", [1, HID], F16)
    w1_d = di("w1", [P, 2 * 2 * HID], F16)
    b1_d = di("b1", [1, 2 * HID], F16)
    w2_d = di("w2", [P, 4 * OUT_DIM], F16)
    b2_d = di("b2", [1, OUT_DIM], F16)
    wd = {}
    for t in range(2):
        wd[f"wkv{t}"] = di(f"wkv{t}", [P, 2 * 2 * HID], F16)
        wd[f"bkv{t}"] = di(f"bkv{t}", [1, 2 * HID], F16)
        wd[f"wq{t}"] = di(f"wq{t}", [P, 2 * HID], F16)
        wd[f"bq{t}"] = di(f"bq{t}", [1, HID], F16)
        wd[f"wsk{t}"] = di(f"wsk{t}", [P, 2 * HID], F16)
        wd[f"bsk{t}"] = di(f"bsk{t}", [1, HID], F16)
        wd[f"wg{t}"] = di(f"wg{t}", [P, 2 * HID], F16)
        wd[f"bg{t}"] = di(f"bg{t}", [1, HID], F16)
        wd[f"atts{t}"] = di(f"atts{t}", [P, HID], F16)
        wd[f"attd{t}"] = di(f"attd{t}", [P, HID], F16)
    if not ln_trivial:
        for i in range(4):
            wd[f"lng{i}"] = di(f"lng{i}", [P, HID], F16)
            wd[f"lnb{i}"] = di(f"lnb{i}", [P, HID], F16)

    out_d = nc.dram_tensor("out", [N_GRAPHS, OUT_DIM], F32, kind="ExternalOutput")
    dump_d = {}
    if DEBUG_DUMPS:
        for l in range(-1, 4):
            dump_d[l] = nc.dram_tensor(
                f"hdump{l}", [P, NB * HID], F32, kind="ExternalOutput"
            )

    h_all = nc.alloc_sbuf_tensor("h_all", [P, NB * HID], F32)
    hT_all = nc.alloc_sbuf_tensor("hT_all", [P, 2 * NPC], F16)
    q_all = nc.alloc_sbuf_tensor("q_all", [P, NB * HID], F16)
    ad_all = nc.alloc_sbuf_tensor("ad_all", [P, NB * HEADS], F16)
    xT_sb = nc.alloc_sbuf_tensor("xT_sb", [IN_DIM, NPC], F16)
    kvidx_sb = nc.alloc_sbuf_tensor("kvidx_sb", [P, NB * NTT * 8], I16)
    gatidx_sb = nc.alloc_sbuf_tensor("gatidx_sb", [P, NB * NTG * 8], I16)

    SQ32 = 1.0 / float(np.sqrt(HDIM))
    QCH = 6  # j-tiles per PSUM chunk in the per-edge q expansion (3 banks)

    with tile.TileContext(nc) as tc:
        with (
            tc.tile_pool(name="wpool", bufs=1) as wp,
            tc.tile_pool(name="spool", bufs=2) as sp,
            tc.tile_pool(name="gpool", bufs=3) as gp,
            tc.tile_pool(name="g2pool", bufs=2) as gp2,
            tc.tile_pool(name="psA", bufs=1, space="PSUM") as psA,
            tc.tile_pool(name="psB", bufs=1, space="PSUM") as psB,
            tc.tile_pool(name="psQ", bufs=1, space="PSUM") as psQ,
            tc.tile_pool(name="psT", bufs=1, space="PSUM") as psT,
            tc.tile_pool(name="psC", bufs=1, space="PSUM") as psC,
            tc.tile_pool(name="dram", bufs=1, space="DRAM") as dp,
        ):
            nc.sync.dma_start(xT_sb.ap(), xT_d.ap())
            nc.sync.dma_start(kvidx_sb.ap(), kvidx_d.ap())
            nc.sync.dma_start(gatidx_sb.ap(), gatidx_d.ap())

            ident = wp.tile([P, P], F16, tag="ident")
            make_identity(nc, ident[:])
            ones1 = wp.tile([1, P], F16, tag="ones1")
            nc.vector.memset(ones1[:], 1.0)
            eps_t = wp.tile([P, 1], F32, tag="eps")
            nc.vector.memset(eps_t[:], LN_EPS)
            # seed for the softmax denominator: 1e-4 * 1e-4 -> 1e-8 in PSUM
            epsA = wp.tile([1, P], F16, tag="epsA")
            nc.vector.memset(epsA[:], 1e-4)
            epsrow = wp.tile([1, HID + HEADS], F16, tag="epsrow")
            nc.vector.memset(epsrow[:], 1e-4)

            def load_w(d, shape, tag, dt=F16):
                t = wp.tile(shape, dt, tag=tag)
                nc.sync.dma_start(t[:], d.ap())
                return t

            win_t = load_w(win_d, [IN_DIM, HID], "win")
            bin_t = load_w(bin_d, [1, HID], "b_in")
            w1_t = load_w(w1_d, [P, 2 * 2 * HID], "w1")
            b1_t = load_w(b1_d, [1, 2 * HID], "b1")
            w2_t = load_w(w2_d, [P, 4 * OUT_DIM], "w2")
            b2_t = load_w(b2_d, [1, OUT_DIM], "b2")
            wt = {}
            for t in range(2):
                for nm, sh in [
                    (f"wkv{t}", [P, 2 * 2 * HID]),
                    (f"bkv{t}", [1, 2 * HID]),
                    (f"wq{t}", [P, 2 * HID]),
                    (f"bq{t}", [1, HID]),
                    (f"wsk{t}", [P, 2 * HID]),
                    (f"bsk{t}", [1, HID]),
                    (f"wg{t}", [P, 2 * HID]),
                    (f"bg{t}", [1, HID]),
                    (f"atts{t}", [P, HID]),
                    (f"attd{t}", [P, HID]),
                ]:
                    wt[nm] = load_w(wd[nm], sh, nm)
            if not ln_trivial:
                for i in range(4):
                    wt[f"lng{i}"] = load_w(wd[f"lng{i}"], [P, HID], f"lng{i}")
                    wt[f"lnb{i}"] = load_w(wd[f"lnb{i}"], [P, HID], f"lnb{i}")
            invcnt_t = load_w(invcnt_d, [N_GRAPHS, 1], "invcnt", F32)
            zrow_t = load_w(zeros_d, [1, 2 * HID], "zrow")
            Sp_sb = []
            for b in range(NB):
                spt = wp.tile([P, N_GRAPHS], F8, tag=f"S_p{b}")
                nc.sync.dma_start(spt[:], Sp_d.ap()[b * P : (b + 1) * P, :])
                Sp_sb.append(spt)

            def mm_dense(psum, lhsT0, lhsT1, w_tile, ncols, bias_tile):
                nc.tensor.matmul(
                    psum, lhsT=lhsT0, rhs=w_tile[:, 0:ncols], start=True, stop=False
                )
                nc.tensor.matmul(
                    psum, lhsT=lhsT1, rhs=w_tile[:, ncols : 2 * ncols],
                    start=False, stop=False,
                )
                nc.tensor.matmul(
                    psum, lhsT=ones1[:], rhs=bias_tile[:, 0:ncols],
                    start=False, stop=True,
                )

            def hT_slices(b):
                l0 = hT_all.ap()[:, 0 * NPC + b * P : 0 * NPC + (b + 1) * P]
                l1 = hT_all.ap()[:, 1 * NPC + b * P : 1 * NPC + (b + 1) * P]
                return l0, l1

            def store_h_and_hT(src_sb_f32, b):
                h16 = sp.tile([P, HID], F16, tag="h16")
                nc.scalar.activation(h16[:], src_sb_f32, AF.Copy)
                for f in range(2):
                    ptp = psT.tile([P, P], F16, space="PSUM", tag="ptp")
                    nc.tensor.transpose(ptp[:], h16[:, f * P : (f + 1) * P], ident[:])
                    nc.scalar.activation(
                        hT_all.ap()[:, f * NPC + b * P : f * NPC + (b + 1) * P],
                        ptp[:],
                        AF.Copy,
                    )
                return h16

            HA = HAB * P
            HB = NPC - HA
            kv_tab = dp.tile([TABN, 2 * HID], F16, tag="kv_tab")
            gat_tab = dp.tile([TABN, GATW], F16, tag="gat_tab")
            kv_bncA = dp.tile([HA, 2 * HID], F16, tag="kv_bncA")
            kv_bncB = dp.tile([HB, 2 * HID], F16, tag="kv_bncB")
            gat_bncA = dp.tile([HA, GATW], F16, tag="gat_bncA")
            gat_bncB = dp.tile([HB, GATW], F16, tag="gat_bncB")
            pool_in = dp.tile([N_GRAPHS, HID], F32, tag="pool_in")
            pool_out = dp.tile([N_GRAPHS, HID], F32, tag="pool_out")
            # constant zero pad row of each table, written once
            nc.sync.dma_start(kv_tab[TABN - 1 : TABN, :], zrow_t[:])
            nc.sync.dma_start(
                gat_tab[TABN - 1 : TABN, 0:GATW], zrow_t[:, 0:GATW]
            )

            psum_pool = psC.tile([N_GRAPHS, HID], F32, space="PSUM", tag="ps_pool")

            def dense_phase(layer, b):
                """Dense matmuls producing layer `layer`'s tables for block b.
                Src-side results go to DRAM staging (kv_bnc/gat_bnc) for the
                AllGather; dst-side (q/a_d) stay in SBUF."""
                l0, l1 = hT_slices(b)
                if b < HAB:
                    kv_bnc, gat_bnc = kv_bncA, gat_bncA
                    rows = slice(b * P, (b + 1) * P)
                else:
                    kv_bnc, gat_bnc = kv_bncB, gat_bncB
                    rows = slice((b - HAB) * P, (b - HAB + 1) * P)
                t = layer // 2
                if layer % 2 == 0:  # TransformerConv: kv (global) + q (local)
                    ps = psA.tile([P, 2 * HID], F32, space="PSUM", tag="ps_dense")
                    mm_dense(ps[:], l0, l1, wt[f"wkv{t}"], 2 * HID, wt[f"bkv{t}"])
                    kv16 = sp.tile([P, 2 * HID], F16, tag="kv16")
                    nc.scalar.activation(kv16[:], ps[:], AF.Copy)
                    nc.sync.dma_start(kv_bnc[rows, :], kv16[:])
                    ps2 = psB.tile([P, 2 * HID], F32, space="PSUM", tag="ps_b")
                    mm_dense(ps2[:, 0:HID], l0, l1, wt[f"wq{t}"], HID, wt[f"bq{t}"])
                    nc.scalar.activation(
                        q_all.ap()[:, b * HID : (b + 1) * HID], ps2[:, 0:HID],
                        AF.Copy,
                    )
                else:  # GATConv: hh (global), a_dst (local)
                    ps = psA.tile([P, 2 * HID], F32, space="PSUM", tag="ps_dense")
                    mm_dense(ps[:, 0:HID], l0, l1, wt[f"wg{t}"], HID, wt[f"bg{t}"])
                    hh16 = sp.tile([P, HID], F16, tag="hh16")
                    nc.scalar.activation(hh16[:], ps[:, 0:HID], AF.Copy)
                    nc.sync.dma_start(gat_bnc[rows, :], hh16[:])
                    proda = sp.tile([P, HID], F16, tag="prodA")
                    nc.vector.tensor_tensor(
                        out=proda[:], in0=hh16[:], in1=wt[f"attd{t}"][:],
                        op=mybir.AluOpType.mult,
                    )
                    asum = sp.tile([P, HEADS], F32, tag="asum")
                    nc.vector.tensor_reduce(
                        out=asum[:],
                        in_=proda[:].rearrange("p (h w) -> p h w", h=HEADS),
                        axis=mybir.AxisListType.X,
                        op=mybir.AluOpType.add,
                    )
                    nc.scalar.activation(
                        ad_all.ap()[:, b * HEADS : (b + 1) * HEADS],
                        asum[:], AF.Copy,
                    )

            def ag_half(layer, half):
                if layer % 2 == 0:
                    bnc = kv_bncA if half == 0 else kv_bncB
                    tab = kv_tab
                else:
                    bnc = gat_bncA if half == 0 else gat_bncB
                    tab = gat_tab
                lo, hi = (0, C * HA) if half == 0 else (C * HA, C * NPC)
                nc.gpsimd.collective_compute(
                    "AllGather",
                    mybir.AluOpType.bypass,
                    replica_groups=[list(range(C))],
                    ins=[bnc.opt()],
                    outs=[tab[lo:hi, :]],
                )

            # ---- phase 0: h0 = x @ Win + b_in, then layer-0 dense ----
            for b in range(NB):
                # psQ is idle in phase 0; using it here decouples the h0
                # chain from the dense-phase psA/psB ping-pongs
                ps = psQ.tile([P, 2 * HID], F32, space="PSUM", tag="ps_q")
                nc.tensor.matmul(
                    ps[:, 0:HID], lhsT=xT_sb.ap()[:, b * P : (b + 1) * P],
                    rhs=win_t[:], start=True, stop=False,
                )
                nc.tensor.matmul(
                    ps[:, 0:HID], lhsT=ones1[:], rhs=bin_t[:], start=False, stop=True
                )
                nc.scalar.activation(
                    h_all.ap()[:, b * HID : (b + 1) * HID], ps[:, 0:HID], AF.Copy
                )
                store_h_and_hT(h_all.ap()[:, b * HID : (b + 1) * HID], b)
                dense_phase(0, b)
                if b == HAB - 1:
                    ag_half(0, 0)
            ag_half(0, 1)
            if DEBUG_DUMPS:
                nc.sync.dma_start(dump_d[-1].ap(), h_all.ap())

            for layer in range(4):
                is_t = layer % 2 == 0
                t = layer // 2
                NT = NTT if is_t else NTG
                idx_sb = kvidx_sb if is_t else gatidx_sb
                tab = kv_tab if is_t else gat_tab
                S_d = St_d if is_t else Sg_d
                ST_d = STt_d if is_t else STg_d
                WROW = 2 * HID if is_t else GATW

                g_kv2 = None
                for b in range(NB):
                    NE = NT * P
                    if is_t:
                        isl = slice(b * NT * 8, (b + 1) * NT * 8)
                        g_kv = gp.tile([P, NTMAX, WROW], F16, tag="g_big")
                        nc.gpsimd.dma_gather(
                            g_kv[:, 0:NT, :], tab[:], idx_sb.ap()[:, isl],
                            NE, NE, WROW, single_packet=False,
                        )
                        gsl = g_kv[:, 0:NT, :]
                    else:
                        # GAT rows are half-width: gather TWO blocks per
                        # instruction into the same-size pool buffer, halving
                        # per-gather fixed cost on the train
                        if b % 2 == 0:
                            isl = slice(b * NT * 8, (b + 2) * NT * 8)
                            g_kv2 = gp.tile(
                                [P, 2 * NTMAX, WROW], F16, tag="g_big"
                            )
                            nc.gpsimd.dma_gather(
                                g_kv2[:, 0 : 2 * NT, :], tab[:],
                                idx_sb.ap()[:, isl],
                                2 * NE, 2 * NE, WROW, single_packet=False,
                            )
                        gsl = g_kv2[:, (b % 2) * NT : (b % 2 + 1) * NT, :]
                    S_sb = gp2.tile([P, NTMAX * P], F8, tag="S_sb")
                    nc.sync.dma_start(
                        S_sb[:, 0 : NT * P], S_d.ap()[b * P : (b + 1) * P, :]
                    )
                    ST_sb = gp2.tile([P, NTMAX * P], F8, tag="ST_sb")
                    nc.sync.dma_start(
                        ST_sb[:, 0 : NT * P], ST_d.ap()[b * P : (b + 1) * P, :]
                    )

                    rhs = gp.tile([P, NTMAX, HID + HEADS], F16, tag="rhs")
                    red = gp2.tile([P, NTMAX * HEADS], F32, tag="red")
                    expdst = rhs[:, 0:NT, HID : HID + HEADS]
                    if is_t:
                        # per-edge q via one-hot transposed matmuls, chunked
                        # through PSUM; fused with k into the kq product
                        for j0 in range(0, NT, QCH):
                            j1 = min(j0 + QCH, NT)
                            psq = psQ.tile(
                                [P, QCH * HID], F32, space="PSUM", tag="ps_q"
                            )
                            for j in range(j0, j1):
                                nc.tensor.matmul(
                                    psq[:, (j - j0) * HID : (j - j0 + 1) * HID],
                                    lhsT=ST_sb[:, j * P : (j + 1) * P],
                                    rhs=q_all.ap()[:, b * HID : (b + 1) * HID],
                                    start=True, stop=True,
                                )
                            nc.vector.tensor_tensor(
                                out=rhs[:, j0:j1, 0:HID],
                                in0=gsl[:, j0:j1, 0:HID],
                                in1=psq[:, 0 : (j1 - j0) * HID].rearrange(
                                    "p (t d) -> p t d", d=HID
                                ),
                                op=mybir.AluOpType.mult,
                            )
                        nc.vector.tensor_reduce(
                            out=red[:, 0 : NT * HEADS],
                            in_=rhs[:, 0:NT, 0:HID].rearrange(
                                "p t (h w) -> p t h w", h=HEADS
                            ),
                            axis=mybir.AxisListType.X,
                            op=mybir.AluOpType.add,
                        )
                        nc.scalar.activation(
                            expdst,
                            red[:, 0 : NT * HEADS].rearrange(
                                "p (t h) -> p t h", h=HEADS
                            ),
                            AF.Exp,
                            scale=SQ32,
                        )
                        vpart = gsl[:, :, HID : 2 * HID]
                    else:
                        # per-edge a_src = <hh_src, att_src> on DVE (att_src
                        # broadcast along j-tiles); a_dst via one-hot
                        # transposed matmuls (all j-tiles fit one PSUM bank)
                        psad = psQ.tile(
                            [P, NTMAX * HEADS], F32, space="PSUM", tag="ps_q"
                        )
                        for j in range(NT):
                            nc.tensor.matmul(
                                psad[:, j * HEADS : (j + 1) * HEADS],
                                lhsT=ST_sb[:, j * P : (j + 1) * P],
                                rhs=ad_all.ap()[:, b * HEADS : (b + 1) * HEADS],
                                start=True, stop=True,
                            )
                        nc.vector.tensor_tensor(
                            out=rhs[:, 0:NT, 0:HID],
                            in0=gsl,
                            in1=wt[f"atts{t}"][:]
                            .rearrange("p (o c) -> p o c", o=1)
                            .to_broadcast([P, NT, HID]),
                            op=mybir.AluOpType.mult,
                        )
                        asr = gp2.tile([P, NTMAX * HEADS], F32, tag="asr")
                        nc.vector.tensor_reduce(
                            out=asr[:, 0 : NT * HEADS],
                            in_=rhs[:, 0:NT, 0:HID].rearrange(
                                "p t (h w) -> p t h w", h=HEADS
                            ),
                            axis=mybir.AxisListType.X,
                            op=mybir.AluOpType.add,
                        )
                        esum = gp2.tile([P, NTMAX * HEADS], F16, tag="esum")
                        nc.vector.tensor_tensor(
                            out=esum[:, 0 : NT * HEADS],
                            in0=asr[:, 0 : NT * HEADS],
                            in1=psad[:, 0 : NT * HEADS],
                            op=mybir.AluOpType.add,
                        )
                        nc.scalar.activation(
                            red[:, 0 : NT * HEADS],
                            esum[:, 0 : NT * HEADS],
                            AF.Prelu,
                            alpha=0.2,
                        )
                        nc.scalar.activation(
                            expdst,
                            red[:, 0 : NT * HEADS].rearrange(
                                "p (t h) -> p t h", h=HEADS
                            ),
                            AF.Exp,
                        )
                        vpart = gsl[:, :, 0:HID]
                    nc.vector.tensor_tensor(
                        out=rhs[:, 0:NT, 0:HID].rearrange(
                            "p t (h w) -> p t h w", h=HEADS
                        ),
                        in0=vpart.rearrange("p t (h w) -> p t h w", h=HEADS),
                        in1=expdst.to_broadcast([P, NT, HEADS, HDIM]),
                        op=mybir.AluOpType.mult,
                    )
                    ps_agg = psB.tile(
                        [P, HID + HEADS], F32, space="PSUM", tag="ps_agg"
                    )
                    nc.tensor.matmul(
                        ps_agg[:], lhsT=epsA[:], rhs=epsrow[:],
                        start=True, stop=False,
                    )
                    for j in range(NT):
                        nc.tensor.matmul(
                            ps_agg[:],
                            lhsT=S_sb[:, j * P : (j + 1) * P],
                            rhs=rhs[:, j, :],
                            start=False,
                            stop=(j == NT - 1),
                        )
                    l0, l1 = hT_slices(b)
                    ps_skip = psB.tile([P, 2 * HID], F32, space="PSUM", tag="ps_b")
                    if is_t:
                        mm_dense(
                            ps_skip[:, 0:HID], l0, l1, wt[f"wsk{t}"], HID,
                            wt[f"bsk{t}"],
                        )
                    else:
                        nc.tensor.matmul(
                            ps_skip[:, 0:HID], lhsT=ones1[:], rhs=wt[f"bg{t}"][:],
                            start=True, stop=True,
                        )
                    rec = sp.tile([P, HEADS], F32, tag="rec")
                    nc.vector.reciprocal(rec[:], ps_agg[:, HID : HID + HEADS])
                    t1 = sp.tile([P, HID], F32, tag="t1")
                    nc.vector.tensor_tensor(
                        out=t1[:].rearrange("p (h w) -> p h w", h=HEADS),
                        in0=ps_agg[:, 0:HID].rearrange("p (h w) -> p h w", h=HEADS),
                        in1=rec[:].to_broadcast([P, HEADS, HDIM]),
                        op=mybir.AluOpType.mult,
                    )
                    t2 = sp.tile([P, HID], F32, tag="t2")
                    nc.vector.tensor_tensor(
                        out=t2[:], in0=t1[:], in1=ps_skip[:, 0:HID],
                        op=mybir.AluOpType.add,
                    )
                    if not is_t:
                        nc.vector.tensor_tensor(
                            out=t2[:], in0=t2[:],
                            in1=h_all.ap()[:, b * HID : (b + 1) * HID],
                            op=mybir.AluOpType.add,
                        )
                    mus = sp.tile([P, 1], F32, tag="mus")
                    nc.vector.reduce_sum(mus[:], t2[:], axis=mybir.AxisListType.X)
                    negmu = sp.tile([P, 1], F32, tag="negmu")
                    nc.scalar.activation(
                        negmu[:], mus[:], AF.Copy, scale=-1.0 / HID
                    )
                    t2c = sp.tile([P, HID], F32, tag="t2c")
                    nc.scalar.activation(
                        t2c[:], t2[:], AF.Identity, bias=negmu[:, 0:1]
                    )
                    sq = sp.tile([P, HID], F32, tag="sq")
                    nc.scalar.activation(sq[:], t2c[:], AF.Square)
                    s2 = sp.tile([P, 1], F32, tag="s2")
                    nc.vector.reduce_sum(s2[:], sq[:], axis=mybir.AxisListType.X)
                    sd = sp.tile([P, 1], F32, tag="sd")
                    nc.scalar.activation(
                        sd[:], s2[:], AF.Sqrt, scale=1.0 / HID, bias=eps_t[:, 0:1]
                    )
                    rs = sp.tile([P, 1], F32, tag="rs")
                    nc.vector.reciprocal(rs[:], sd[:])
                    hdst = h_all.ap()[:, b * HID : (b + 1) * HID]
                    if ln_trivial:
                        nc.scalar.activation(
                            hdst, t2c[:], AF.Relu, scale=rs[:, 0:1]
                        )
                    else:
                        t3 = sp.tile([P, HID], F32, tag="t3")
                        nc.vector.tensor_scalar(
                            out=t3[:], in0=t2c[:], scalar1=rs[:, 0:1],
                            scalar2=None, op0=mybir.AluOpType.mult,
                        )
                        nc.vector.tensor_tensor(
                            out=t3[:], in0=t3[:], in1=wt[f"lng{layer}"][:],
                            op=mybir.AluOpType.mult,
                        )
                        nc.vector.tensor_tensor(
                            out=t3[:], in0=t3[:], in1=wt[f"lnb{layer}"][:],
                            op=mybir.AluOpType.add,
                        )
                        nc.scalar.activation(hdst, t3[:], AF.Relu)
                    if layer == 3:
                        # last layer: h feeds only the pool matmul; skip the
                        # hT transposes (no next dense phase consumes them)
                        h16 = sp.tile([P, HID], F16, tag="h16")
                        nc.scalar.activation(h16[:], hdst, AF.Copy)
                    else:
                        h16 = store_h_and_hT(hdst, b)
                    if layer == 3:
                        nc.tensor.matmul(
                            psum_pool[:],
                            lhsT=Sp_sb[b][:],
                            rhs=h16[:],
                            start=(b == 0),
                            stop=(b == NB - 1),
                        )
                    else:
                        dense_phase(layer + 1, b)
                        if b == HAB + 2:
                            ag_half(layer + 1, 0)
                if layer < 3:
                    ag_half(layer + 1, 1)
                if DEBUG_DUMPS:
                    nc.sync.dma_start(dump_d[layer].ap(), h_all.ap())

            # ---- pool + MLP ----
            pool_sb = sp.tile([N_GRAPHS, HID], F32, tag="pool_sb")
            nc.scalar.activation(pool_sb[:], psum_pool[:], AF.Copy)
            nc.sync.dma_start(pool_in[:], pool_sb[:])
            nc.gpsimd.collective_compute(
                "AllReduce",
                mybir.AluOpType.add,
                replica_groups=[list(range(C))],
                ins=[pool_in.opt()],
                outs=[pool_out.opt()],
            )
            sums = sp.tile([N_GRAPHS, HID], F32, tag="sums")
            nc.sync.dma_start(sums[:], pool_out[:])
            pooled = sp.tile([N_GRAPHS, HID], F32, tag="pooled")
            nc.vector.tensor_scalar(
                out=pooled[:], in0=sums[:], scalar1=invcnt_t[:, 0:1],
                scalar2=None, op0=mybir.AluOpType.mult,
            )
            p16 = sp.tile([N_GRAPHS, HID], F16, tag="p16")
            nc.scalar.activation(p16[:], pooled[:], AF.Copy)
            pT = sp.tile([P, 2 * N_GRAPHS], F16, tag="pT")
            for f in range(2):
                ptp = psT.tile([P, P], F16, space="PSUM", tag="ptp")
                nc.tensor.transpose(ptp[:], p16[:, f * P : (f + 1) * P], ident[:])
                nc.scalar.activation(
                    pT[:, f * N_GRAPHS : (f + 1) * N_GRAPHS], ptp[:], AF.Copy
                )
            ps1 = psA.tile([P, 2 * HID], F32, space="PSUM", tag="ps_dense")
            nc.tensor.matmul(
                ps1[:], lhsT=pT[:, 0:N_GRAPHS], rhs=w1_t[:, 0 : 2 * HID],
                start=True, stop=False,
            )
            nc.tensor.matmul(
                ps1[:], lhsT=pT[:, N_GRAPHS : 2 * N_GRAPHS],
                rhs=w1_t[:, 2 * HID : 4 * HID], start=False, stop=False,
            )
            nc.tensor.matmul(
                ps1[:], lhsT=ones1[:], rhs=b1_t[:], start=False, stop=True
            )
            h1 = sp.tile([N_GRAPHS, 2 * HID], F16, tag="h1")
            nc.scalar.activation(h1[:], ps1[:], AF.Relu)
            h1T = sp.tile([P, 4 * N_GRAPHS], F16, tag="h1T")
            for f in range(4):
                ptp = psT.tile([P, P], F16, space="PSUM", tag="ptp")
                nc.tensor.transpose(ptp[:], h1[:, f * P : (f + 1) * P], ident[:])
                nc.scalar.activation(
                    h1T[:, f * N_GRAPHS : (f + 1) * N_GRAPHS], ptp[:], AF.Copy
                )
            ps2 = psB.tile([P, 2 * HID], F32, space="PSUM", tag="ps_b")
            for f in range(4):
                nc.tensor.matmul(
                    ps2[:, 0:OUT_DIM],
                    lhsT=h1T[:, f * N_GRAPHS : (f + 1) * N_GRAPHS],
                    rhs=w2_t[:, f * OUT_DIM : (f + 1) * OUT_DIM],
                    start=(f == 0),
                    stop=False,
                )
            nc.tensor.matmul(
                ps2[:, 0:OUT_DIM], lhsT=ones1[:], rhs=b2_t[:], start=False, stop=True
            )
            out_sb = sp.tile([N_GRAPHS, OUT_DIM], F32, tag="out_sb")
            nc.scalar.activation(out_sb[:], ps2[:, 0:OUT_DIM], AF.Copy)
            nc.sync.dma_start(out_d.ap(), out_sb[:])

    nc.compile()
    return nc


_CACHE = {}


def kernel(**inputs):
    meta, in_maps = host_prep(inputs)
    key = tuple(sorted(meta.items()))
    if key not in _CACHE:
        _CACHE[key] = build_program(meta)
    nc = _CACHE[key]
    res = bass_utils.run_bass_kernel_spmd(nc, in_maps, core_ids=list(range(C)))
    return np.asarray(res.results[0]["out"], np.float32)
